# revision 60
# baseline (speedup 1.0000x reference)
"""Multi-head attention with additive positional bias on 8 Trainium2 cores.

Problem: q,k,v [8, 1024, 512] fp32, pos_bias [1, 8, 1024, 1024] fp32,
8 heads x head_dim 64, out = softmax(q@k^T * scale + bias) @ v.

Sharding: one head per NeuronCore (tensor parallel over heads).

Per-core pipeline (scores transposed: S^T[j,i], j on partitions; q is
pre-scaled by A*SCALE on the host so psum holds A*s where A=128/ln2).
The softmax exp drain is a hard constraint (1M psum f32 values per batch
must leave PSUM through ScalarE/VectorE at 1 elem/cycle/lane) and the PE
paces the steady state (~95% busy), so the work is split:
  - PE: QK^T bf16 per pair with column chunks interleaved across the K=64
    row halves [t0c0,t1c0,t0c1,t1c1] so both chunks co-execute; previous
    batch's PV chunks follow each pair's mm1 (mm1 first: ScalarE's exp
    chain needs those psums earliest).
  - VectorE: Schraudolph bit-trick exp for j-tiles 0,2,4,6:
    bitcast_bf16(int16(A*s + 16249 + A*b)) via one scalar_tensor_tensor
    against a resident bf16 A*b table, plus deferred exp(bias) multiplies
    for tiles 5,7 (bf16 tensor_tensor hits the DVE 2x_1P mode, ~683ns;
    emitted after the batch's STTs so the psum drains never queue behind
    ScalarE-dependent work).
  - ScalarE: true exp (free affine scale=1/A) for tiles 1,3,5,7 + the po
    evacuation, split into two 512-col halves (bank A mid-batch, bank B at
    the tail) so only a half-evac sits on the critical loop.
  - GpSimd: exp(bias) multiply for tiles 1,3 (GpSimd TT is ~2.1us/tile;
    these P tiles are consumed late by the next batch's PV).
  - PV: po[dv,i] += [V|ones]^T @ P^T, bank-split: pairs 0-1 accumulate po
    cols 0:512 over all 8 j-tiles (order 0,2,4,6 then 1,3 GpSimd-muled
    then 5,7 Vector-muled, matching P availability), pairs 2-3 cols
    512:1024; 65th row = softmax denominators via the ones column; host
    divides and untransposes.
Even j-tiles on VectorE matter: the 3-deep psum rotation makes mm1(t) wait
on drain(t-3), and even tiles put a Vector STT (not a ScalarE exp that
queues behind the evac) on that edge.
HAM: 8 full-size warmup matmuls during the prologue DMAs + 2 per batch-0
pair keep the PE activity window busy so the clock gate hits 8/8 (~13us)
instead of ~24us; small matmuls do not register as busy.
DMA diet: k is parity-packed [128,512] (even tiles rows 0:64, odd 64:128 -
no duplication), vp carries only 66 columns, output is bf16.
Table DMAs are emitted in first-use order behind batch 0's q/k so the
prologue overlaps compute (the single sync DMA ring is serial; the ACT
ring regressed when tried).
"""

import numpy as np
from contextlib import ExitStack

import concourse.bacc as bacc
import concourse.bass as bass
import concourse.mybir as mybir
import concourse.tile as tile
from concourse.bass_utils import run_bass_kernel_spmd

B = 8          # batch
S = 1024       # sequence length
D = 512        # model dim
H = 8          # heads
HD = 64        # head dim
NT = S // 128  # 128-row j-tiles per sequence
SCALE = HD ** -0.5

A_SCH = 128.0 / np.log(2.0)          # Schraudolph scale (bf16: 2^7 mantissa)
DELTA = -7.0                         # minimax-ish centering of the bit trick
SCH_TILES = (0, 2, 4, 6)             # j-tiles drained by the DVE bit-trick exp
GPS_MUL_TILES = (1, 3)               # exp-tiles whose bias-mul runs on GpSimd
MM2_ORDER = (0, 2, 4, 6, 1, 3, 5, 7)  # PV order: STT tiles first, then the
                                      # GpSimd-muled, then the Vector-muled
                                      # (V muls run after its STTs, latest)
WARMUP_MM = 8                        # full-size PE warmup matmuls during the
                                     # prologue DMA (~3.4us of PE busy: trips
                                     # the HAM activity window so the clock
                                     # gate is 8/8 when real work starts)

_PROGRAM = None


def _emit(ctx, tc, out, qt, kt, vp, bb, eb):
    nc = tc.nc
    f32 = mybir.dt.float32
    bf16 = mybir.dt.bfloat16
    i16 = mybir.dt.int16

    singles = ctx.enter_context(tc.tile_pool(name="singles", bufs=1))
    qk_pool = ctx.enter_context(tc.tile_pool(name="qk_pool", bufs=3))
    v_pool = ctx.enter_context(tc.tile_pool(name="v_pool", bufs=3))
    eg_pool = ctx.enter_context(tc.tile_pool(name="eg_pool", bufs=8))
    ev_pool = ctx.enter_context(tc.tile_pool(name="ev_pool", bufs=8))
    p_pool = ctx.enter_context(tc.tile_pool(name="p_pool", bufs=20))
    o_pool = ctx.enter_context(tc.tile_pool(name="o_pool", bufs=4))
    ps_s = ctx.enter_context(tc.tile_pool(name="ps_s", bufs=3, space="PSUM"))
    ps_o = ctx.enter_context(tc.tile_pool(name="ps_o", bufs=1, space="PSUM"))

    # batch-0 inputs first so compute starts immediately; then tables in
    # first-use order so batch 0's drains are fed as they arrive
    qtb0 = qk_pool.tile([128, S], bf16, name="qtb0", tag="qtb")
    nc.sync.dma_start(out=qtb0, in_=qt[0])
    ktb0 = qk_pool.tile([128, S // 2], bf16, name="ktb0", tag="ktb")
    nc.sync.dma_start(out=ktb0, in_=kt[0])
    bb_tiles = [None] * NT
    eb_tiles = {}
    for t in range(NT):
        if t in SCH_TILES:
            bbt = singles.tile([128, S], bf16, name=f"bbt{t}")
            nc.sync.dma_start(out=bbt, in_=bb[t * 128:(t + 1) * 128, :])
            bb_tiles[t] = bbt
        else:
            ebt = singles.tile([128, S], bf16, name=f"ebt{t}")
            nc.sync.dma_start(out=ebt, in_=eb[t * 128:(t + 1) * 128, :])
            eb_tiles[t] = ebt
    vpb0 = v_pool.tile([128, NT, 66], bf16, name="vpb0", tag="vpb")
    nc.sync.dma_start(out=vpb0, in_=vp[0])

    # PE warmup: FULL-SIZE matmuls (K=128, N=512 — small ones don't register
    # as "busy" to the HAM activity monitor) keep the PE busy during the
    # prologue DMAs and through batch 0 (which has no PV to interleave), so
    # the HAM clock-gate reaches 8/8 before the steady state. The warm psum
    # lives in the ps_o pool: it is unused until batch 1 allocates po, and
    # batch 1's first PV clears the bank (start=True) anyway.
    warm = singles.tile([128, 512 + 128], bf16, name="warm")
    nc.vector.memset(warm, 0.0)
    wps = ps_o.tile([128, S], f32, tag="po")

    def warm_mms(n):
        for _ in range(n):
            nc.tensor.matmul(
                wps[:, 0:512], warm[:, 0:128], warm[:, 128:640],
                start=True, stop=True,
            )

    warm_mms(WARMUP_MM)

    prev = None  # (ptiles, vpb) of previous batch, for interleaved mm2

    def mm1_pair(qtb, ktb, ps0, ps1, t0, t1, serial=False):
        """psum = A*s for j-tiles t0 (rows 0:64) and t1 (rows 64:128).

        Column chunks are interleaved [t0c0, t1c0, t0c1, t1c1]: matmul
        starts are pc-monotone, so same-row-group chunks back-to-back
        would serialize; alternating row groups lets each chunk pair
        co-execute (4 matmuls in ~2 matmul times instead of 3).
        serial=True keeps all of t0 before t1 (batch 0 pair 0: t1's hi-row
        inputs arrive later, and co-execution would head-of-line block t0).
        """
        if serial:
            order = ((t0, ps0, 0), (t0, ps0, 1), (t1, ps1, 0), (t1, ps1, 1))
        else:
            order = ((t0, ps0, 0), (t1, ps1, 0), (t0, ps0, 1), (t1, ps1, 1))
        for t, ps, c in order:
            cs = slice(c * 512, (c + 1) * 512)
            half, idx = t % 2, t // 2
            r = slice(half * 64, (half + 1) * 64)
            nc.tensor.matmul(
                ps[:, cs], ktb[r, idx * 128:(idx + 1) * 128], qtb[r, cs],
                start=True, stop=True,
            )

    pending_vmuls = []  # deferred (pt, et, t): emitted after the batch's STTs

    def drain(ps, t, vmul_all=False):
        """psum -> P tile (bf16 ~ exp(s+b))."""
        pt = p_pool.tile([128, S], bf16, tag="pt")
        if t in SCH_TILES:
            # int16(A*s + (16256-7) + A*b) bitcast to bf16 = ~exp(s+b)
            nc.vector.scalar_tensor_tensor(
                pt.bitcast(i16),
                ps,
                float(127.0 * 128.0 + DELTA),
                bb_tiles[t],
                mybir.AluOpType.add,
                mybir.AluOpType.add,
            )
        else:
            # separate et pools per mul engine: a shared pool would make
            # ScalarE's exp wait on GpSimd's slow muls for tile recycling
            if t in GPS_MUL_TILES and not vmul_all:
                et = eg_pool.tile([128, S], bf16, tag="etg")
                nc.scalar.activation(
                    et, ps, mybir.ActivationFunctionType.Exp,
                    scale=float(1.0 / A_SCH),
                )
                nc.gpsimd.tensor_mul(pt, et, eb_tiles[t])
            else:
                et = ev_pool.tile([128, S], bf16, tag="etv")
                nc.scalar.activation(
                    et, ps, mybir.ActivationFunctionType.Exp,
                    scale=float(1.0 / A_SCH),
                )
                # defer: keeps VectorE's queue free for the psum-draining
                # STTs (a mul here would make them wait on ScalarE's exp)
                pending_vmuls.append((pt, et, t))
        return pt

    def mm2(po, ptiles, vpb, oi, c):
        """One 512-col PV chunk: bank c of po accumulates j-tile MM2_ORDER[oi]."""
        t = MM2_ORDER[oi]
        cs = slice(c * 512, (c + 1) * 512)
        nc.tensor.matmul(
            po[:, cs],
            vpb[:, t, 0:65],
            ptiles[t][:, cs],
            start=(oi == 0),
            stop=(oi == NT - 1),
        )

    def finish_half(osb, po, c):
        cs = slice(c * 512, (c + 1) * 512)
        nc.scalar.activation(
            osb[:, cs], po[:, cs], mybir.ActivationFunctionType.Copy
        )

    for b in range(B):
        if b == 0:
            qtb, ktb, vpb = qtb0, ktb0, vpb0
        else:
            qtb = qk_pool.tile([128, S], bf16, tag="qtb")
            nc.sync.dma_start(out=qtb, in_=qt[b])
            ktb = qk_pool.tile([128, S // 2], bf16, tag="ktb")
            nc.sync.dma_start(out=ktb, in_=kt[b])
            vpb = v_pool.tile([128, NT, 66], bf16, tag="vpb")
            nc.sync.dma_start(out=vpb, in_=vp[b])

        po = None
        osb = None
        if prev is not None:
            po = ps_o.tile([65, S], f32, tag="po")
            osb = o_pool.tile([65, S], bf16, tag="osb")
        ptiles = [None] * NT
        for p in range(NT // 2):
            t0, t1 = 2 * p, 2 * p + 1
            # mm1 first: ScalarE's exp chain is the critical path, and its
            # inputs are these psums — the previous batch's PV fills in after
            ps0 = ps_s.tile([128, S], f32, tag="ps")
            ps1 = ps_s.tile([128, S], f32, tag="ps")
            mm1_pair(qtb, ktb, ps0, ps1, t0, t1)
            if prev is not None:
                # bank-split PV: pairs 0-1 fill po bank A (cols 0:512) for all
                # 8 j-tiles, pairs 2-3 fill bank B — bank A's evacuation runs
                # mid-batch, leaving only a half-evac on the loop tail
                c, base = (0, 4 * p) if p < 2 else (1, 4 * (p - 2))
                for oi in range(base, base + 4):
                    mm2(po, prev[0], prev[1], oi, c)
            ptiles[t0] = drain(ps0, t0)
            ptiles[t1] = drain(ps1, t1)
            if prev is not None and p == 1:
                finish_half(osb, po, 0)
            if b == 0:
                # batch 0 is drain/DMA-paced with little PE work: keep the
                # HAM activity window busy so the clock-gate warms early
                warm_mms(2)
        for pt, et, t in pending_vmuls:
            nc.vector.tensor_mul(pt, et, eb_tiles[t])
        pending_vmuls.clear()
        if prev is not None:
            finish_half(osb, po, 1)
            nc.sync.dma_start(out=out[b - 1], in_=osb)
        prev = (ptiles, vpb)

    # last batch's PV: interleave the two bank chunks per j-tile so each runs
    # as soon as its P tile lands (the all-A-then-all-B order would serialize
    # the whole B pass behind the latest-produced P tile)
    po = ps_o.tile([65, S], f32, tag="po")
    osb = o_pool.tile([65, S], bf16, tag="osb")
    for oi in range(NT):
        mm2(po, prev[0], prev[1], oi, 0)
        mm2(po, prev[0], prev[1], oi, 1)
        if oi == NT - 1:
            finish_half(osb, po, 0)
    finish_half(osb, po, 1)
    nc.sync.dma_start(out=out[B - 1], in_=osb)


def _build_program():
    nc = bacc.Bacc("TRN2", target_bir_lowering=False, debug=False)
    bf16 = mybir.dt.bfloat16
    qt = nc.dram_tensor("qt", [B, 128, S], bf16, kind="ExternalInput").ap()
    kt = nc.dram_tensor("kt", [B, 128, S // 2], bf16, kind="ExternalInput").ap()
    vp = nc.dram_tensor("vp", [B, 128, NT, 66], bf16, kind="ExternalInput").ap()
    bb = nc.dram_tensor("bb", [S, S], bf16, kind="ExternalInput").ap()
    eb = nc.dram_tensor("eb", [S, S], bf16, kind="ExternalInput").ap()
    out = nc.dram_tensor(
        "out", [B, 65, S], bf16, kind="ExternalOutput"
    ).ap()
    with tile.TileContext(nc) as tc, ExitStack() as ctx:
        _emit(ctx, tc, out, qt, kt, vp, bb, eb)
    nc.compile()
    return nc


def get_program():
    global _PROGRAM
    if _PROGRAM is None:
        _PROGRAM = _build_program()
    return _PROGRAM


def make_in_maps(q, k, v, pos_bias):
    import ml_dtypes

    nbf16 = ml_dtypes.bfloat16
    q4 = q.reshape(B, S, H, HD)
    k4 = k.reshape(B, S, H, HD)
    v4 = v.reshape(B, S, H, HD)
    qscale = np.float32(SCALE * A_SCH)
    in_maps = []
    for h in range(H):
        qt = np.empty((B, 128, S), nbf16)
        qt[:, :HD, :] = (q4[:, :, h, :].transpose(0, 2, 1) * qscale).astype(nbf16)
        qt[:, HD:, :] = qt[:, :HD, :]
        # k parity-packed: even tiles on rows 0:64, odd tiles on rows 64:128
        kT = k4[:, :, h, :].transpose(0, 2, 1).astype(nbf16)  # [B, 64, S]
        kt = np.empty((B, 128, S // 2), nbf16)
        for t in range(NT):
            half, idx = t % 2, t // 2
            kt[:, half * 64:(half + 1) * 64, idx * 128:(idx + 1) * 128] = (
                kT[:, :, t * 128:(t + 1) * 128]
            )
        vp = np.concatenate(
            [v4[:, :, h, :], np.ones((B, S, 1), np.float32),
             np.zeros((B, S, 1), np.float32)], axis=2
        )  # [B, S, 66]
        vp = np.ascontiguousarray(
            vp.reshape(B, NT, 128, 66).transpose(0, 2, 1, 3)
        ).astype(nbf16)  # [B, 128, NT, 66]
        btT = np.ascontiguousarray(pos_bias[0, h].T).astype(np.float32)  # [j, i]
        bb = (A_SCH * btT).astype(nbf16)
        eb = np.exp(btT).astype(nbf16)
        in_maps.append({"qt": qt, "kt": kt, "vp": vp, "bb": bb, "eb": eb})
    return in_maps


def assemble_output(results):
    out = np.empty((B, S, D), np.float32)
    for h in range(H):
        o = np.asarray(results[h]["out"], np.float32)  # [B, 65, S]
        normed = o[:, :HD, :] / o[:, HD:HD + 1, :]
        out[:, :, h * HD:(h + 1) * HD] = normed.transpose(0, 2, 1)
    return out


def kernel(q, k, v, pos_bias):
    nc = get_program()
    in_maps = make_in_maps(
        np.asarray(q, np.float32),
        np.asarray(k, np.float32),
        np.asarray(v, np.float32),
        np.asarray(pos_bias, np.float32),
    )
    res = run_bass_kernel_spmd(nc, in_maps, list(range(H))).results
    return assemble_output(res)


# revision 61
# speedup vs baseline: 1.0071x; 1.0071x over previous
"""Multi-head attention with additive positional bias on 8 Trainium2 cores.

Problem: q,k,v [8, 1024, 512] fp32, pos_bias [1, 8, 1024, 1024] fp32,
8 heads x head_dim 64, out = softmax(q@k^T * scale + bias) @ v.

Sharding: one head per NeuronCore (tensor parallel over heads).

Per-core pipeline (scores transposed: S^T[j,i], j on partitions; q is
pre-scaled by A*SCALE on the host so psum holds A*s where A=128/ln2).
The softmax exp drain is a hard constraint (1M psum f32 values per batch
must leave PSUM through ScalarE/VectorE at 1 elem/cycle/lane) and the PE
paces the steady state (~95% busy), so the work is split:
  - PE: QK^T bf16 per pair with column chunks interleaved across the K=64
    row halves [t0c0,t1c0,t0c1,t1c1] so both chunks co-execute; previous
    batch's PV chunks follow each pair's mm1 (mm1 first: ScalarE's exp
    chain needs those psums earliest).
  - VectorE: Schraudolph bit-trick exp for j-tiles 0,2,4,6:
    bitcast_bf16(int16(A*s + 16249 + A*b)) via one scalar_tensor_tensor
    against a resident bf16 A*b table, plus deferred exp(bias) multiplies
    for tiles 5,7 (bf16 tensor_tensor hits the DVE 2x_1P mode, ~683ns;
    emitted after the batch's STTs so the psum drains never queue behind
    ScalarE-dependent work).
  - ScalarE: true exp (free affine scale=1/A) for tiles 1,3,5,7 + the po
    evacuation, split into two 512-col halves (bank A mid-batch, bank B at
    the tail) so only a half-evac sits on the critical loop.
  - GpSimd: exp(bias) multiply for tiles 1,3 (GpSimd TT is ~2.1us/tile;
    these P tiles are consumed late by the next batch's PV).
  - PV: po[dv,i] += [V|ones]^T @ P^T, bank-split: pairs 0-1 accumulate po
    cols 0:512 over all 8 j-tiles (order 0,2,4,6 then 1,3 GpSimd-muled
    then 5,7 Vector-muled, matching P availability), pairs 2-3 cols
    512:1024; 65th row = softmax denominators via the ones column; host
    divides and untransposes.
Even j-tiles on VectorE matter: the 3-deep psum rotation makes mm1(t) wait
on drain(t-3), and even tiles put a Vector STT (not a ScalarE exp that
queues behind the evac) on that edge.
HAM: 8 full-size warmup matmuls during the prologue DMAs + 2 per batch-0
pair keep the PE activity window busy so the clock gate hits 8/8 (~13us)
instead of ~24us; small matmuls do not register as busy.
DMA diet: k is parity-packed [128,512] (even tiles rows 0:64, odd 64:128 -
no duplication), vp carries only 66 columns, output is bf16.
Table DMAs are emitted in first-use order behind batch 0's q/k so the
prologue overlaps compute (the single sync DMA ring is serial; the ACT
ring regressed when tried).
"""

import numpy as np
from contextlib import ExitStack

import concourse.bacc as bacc
import concourse.bass as bass
import concourse.mybir as mybir
import concourse.tile as tile
from concourse.bass_utils import run_bass_kernel_spmd

B = 8          # batch
S = 1024       # sequence length
D = 512        # model dim
H = 8          # heads
HD = 64        # head dim
NT = S // 128  # 128-row j-tiles per sequence
SCALE = HD ** -0.5

A_SCH = 128.0 / np.log(2.0)          # Schraudolph scale (bf16: 2^7 mantissa)
DELTA = -7.0                         # minimax-ish centering of the bit trick
SCH_TILES = (0, 2, 4, 6)             # j-tiles drained by the DVE bit-trick exp
GPS_MUL_TILES = (1, 3)               # exp-tiles whose bias-mul runs on GpSimd
MM2_ORDER = (0, 2, 4, 6, 1, 3, 5, 7)  # PV order: STT tiles first, then the
                                      # GpSimd-muled, then the Vector-muled
                                      # (V muls run after its STTs, latest)
WARMUP_MM = 8                        # full-size PE warmup matmuls during the
                                     # prologue DMA (~3.4us of PE busy: trips
                                     # the HAM activity window so the clock
                                     # gate is 8/8 when real work starts)

_PROGRAM = None


def _emit(ctx, tc, out, qt, kt, vp, bb, eb):
    nc = tc.nc
    f32 = mybir.dt.float32
    bf16 = mybir.dt.bfloat16
    i16 = mybir.dt.int16

    singles = ctx.enter_context(tc.tile_pool(name="singles", bufs=1))
    qk_pool = ctx.enter_context(tc.tile_pool(name="qk_pool", bufs=3))
    v_pool = ctx.enter_context(tc.tile_pool(name="v_pool", bufs=3))
    eg_pool = ctx.enter_context(tc.tile_pool(name="eg_pool", bufs=6))
    ev_pool = ctx.enter_context(tc.tile_pool(name="ev_pool", bufs=6))
    p_pool = ctx.enter_context(tc.tile_pool(name="p_pool", bufs=20))
    o_pool = ctx.enter_context(tc.tile_pool(name="o_pool", bufs=4))
    ps_s = ctx.enter_context(tc.tile_pool(name="ps_s", bufs=3, space="PSUM"))
    ps_o = ctx.enter_context(tc.tile_pool(name="ps_o", bufs=1, space="PSUM"))

    # batch-0 inputs first so compute starts immediately; then tables in
    # first-use order so batch 0's drains are fed as they arrive
    qtb0 = qk_pool.tile([128, S], bf16, name="qtb0", tag="qtb")
    nc.sync.dma_start(out=qtb0, in_=qt[0])
    ktb0 = qk_pool.tile([128, S // 2], bf16, name="ktb0", tag="ktb")
    nc.sync.dma_start(out=ktb0, in_=kt[0])
    bb_tiles = [None] * NT
    eb_tiles = {}
    for t in range(NT):
        if t in SCH_TILES:
            bbt = singles.tile([128, S], bf16, name=f"bbt{t}")
            nc.sync.dma_start(out=bbt, in_=bb[t * 128:(t + 1) * 128, :])
            bb_tiles[t] = bbt
        else:
            ebt = singles.tile([128, S], bf16, name=f"ebt{t}")
            nc.sync.dma_start(out=ebt, in_=eb[t * 128:(t + 1) * 128, :])
            eb_tiles[t] = ebt
    vpb0 = v_pool.tile([128, NT, 66], bf16, name="vpb0", tag="vpb")
    nc.sync.dma_start(out=vpb0, in_=vp[0])

    # PE warmup: FULL-SIZE matmuls (K=128, N=512 — small ones don't register
    # as "busy" to the HAM activity monitor) keep the PE busy during the
    # prologue DMAs and through batch 0 (which has no PV to interleave), so
    # the HAM clock-gate reaches 8/8 before the steady state. The warm psum
    # lives in the ps_o pool: it is unused until batch 1 allocates po, and
    # batch 1's first PV clears the bank (start=True) anyway.
    warm = singles.tile([128, 512 + 128], bf16, name="warm")
    nc.vector.memset(warm, 0.0)
    wps = ps_o.tile([128, S], f32, tag="po")

    def warm_mms(n):
        for _ in range(n):
            nc.tensor.matmul(
                wps[:, 0:512], warm[:, 0:128], warm[:, 128:640],
                start=True, stop=True,
            )

    warm_mms(WARMUP_MM)

    prev = None  # (ptiles, vpb) of previous batch, for interleaved mm2

    def mm1_pair(qtb, ktb, ps0, ps1, t0, t1, serial=False):
        """psum = A*s for j-tiles t0 (rows 0:64) and t1 (rows 64:128).

        Column chunks are interleaved [t0c0, t1c0, t0c1, t1c1]: matmul
        starts are pc-monotone, so same-row-group chunks back-to-back
        would serialize; alternating row groups lets each chunk pair
        co-execute (4 matmuls in ~2 matmul times instead of 3).
        serial=True keeps all of t0 before t1 (batch 0 pair 0: t1's hi-row
        inputs arrive later, and co-execution would head-of-line block t0).
        """
        if serial:
            order = ((t0, ps0, 0), (t0, ps0, 1), (t1, ps1, 0), (t1, ps1, 1))
        else:
            order = ((t0, ps0, 0), (t1, ps1, 0), (t0, ps0, 1), (t1, ps1, 1))
        for t, ps, c in order:
            cs = slice(c * 512, (c + 1) * 512)
            half, idx = t % 2, t // 2
            r = slice(half * 64, (half + 1) * 64)
            nc.tensor.matmul(
                ps[:, cs], ktb[r, idx * 128:(idx + 1) * 128], qtb[r, cs],
                start=True, stop=True,
            )

    pending_vmuls = []  # deferred (pt, et, t): emitted after the batch's STTs

    def drain(ps, t, vmul_all=False):
        """psum -> P tile (bf16 ~ exp(s+b))."""
        pt = p_pool.tile([128, S], bf16, tag="pt")
        if t in SCH_TILES:
            # int16(A*s + (16256-7) + A*b) bitcast to bf16 = ~exp(s+b)
            nc.vector.scalar_tensor_tensor(
                pt.bitcast(i16),
                ps,
                float(127.0 * 128.0 + DELTA),
                bb_tiles[t],
                mybir.AluOpType.add,
                mybir.AluOpType.add,
            )
        else:
            # separate et pools per mul engine: a shared pool would make
            # ScalarE's exp wait on GpSimd's slow muls for tile recycling
            if t in GPS_MUL_TILES and not vmul_all:
                et = eg_pool.tile([128, S], bf16, tag="etg")
                nc.scalar.activation(
                    et, ps, mybir.ActivationFunctionType.Exp,
                    scale=float(1.0 / A_SCH),
                )
                nc.gpsimd.tensor_mul(pt, et, eb_tiles[t])
            else:
                et = ev_pool.tile([128, S], bf16, tag="etv")
                nc.scalar.activation(
                    et, ps, mybir.ActivationFunctionType.Exp,
                    scale=float(1.0 / A_SCH),
                )
                # defer: keeps VectorE's queue free for the psum-draining
                # STTs (a mul here would make them wait on ScalarE's exp)
                pending_vmuls.append((pt, et, t))
        return pt

    def mm2(po, ptiles, vpb, oi, c):
        """One 512-col PV chunk: bank c of po accumulates j-tile MM2_ORDER[oi]."""
        t = MM2_ORDER[oi]
        cs = slice(c * 512, (c + 1) * 512)
        nc.tensor.matmul(
            po[:, cs],
            vpb[:, t, 0:65],
            ptiles[t][:, cs],
            start=(oi == 0),
            stop=(oi == NT - 1),
        )

    def finish_half(osb, po, c):
        cs = slice(c * 512, (c + 1) * 512)
        nc.scalar.activation(
            osb[:, cs], po[:, cs], mybir.ActivationFunctionType.Copy
        )

    for b in range(B):
        if b == 0:
            qtb, ktb, vpb = qtb0, ktb0, vpb0
        else:
            qtb = qk_pool.tile([128, S], bf16, tag="qtb")
            nc.sync.dma_start(out=qtb, in_=qt[b])
            ktb = qk_pool.tile([128, S // 2], bf16, tag="ktb")
            nc.sync.dma_start(out=ktb, in_=kt[b])
            vpb = v_pool.tile([128, NT, 66], bf16, tag="vpb")
            nc.sync.dma_start(out=vpb, in_=vp[b])

        po = None
        osb = None
        if prev is not None:
            po = ps_o.tile([65, S], f32, tag="po")
            osb = o_pool.tile([65, S], bf16, tag="osb")
        ptiles = [None] * NT
        for p in range(NT // 2):
            t0, t1 = 2 * p, 2 * p + 1
            # mm1 first: ScalarE's exp chain is the critical path, and its
            # inputs are these psums — the previous batch's PV fills in after
            ps0 = ps_s.tile([128, S], f32, tag="ps")
            ps1 = ps_s.tile([128, S], f32, tag="ps")
            mm1_pair(qtb, ktb, ps0, ps1, t0, t1)
            if prev is not None:
                # bank-split PV: pairs 0-1 fill po bank A (cols 0:512) for all
                # 8 j-tiles, pairs 2-3 fill bank B — bank A's evacuation runs
                # mid-batch, leaving only a half-evac on the loop tail
                c, base = (0, 4 * p) if p < 2 else (1, 4 * (p - 2))
                for oi in range(base, base + 4):
                    mm2(po, prev[0], prev[1], oi, c)
            ptiles[t0] = drain(ps0, t0)
            ptiles[t1] = drain(ps1, t1)
            if prev is not None and p == 1:
                finish_half(osb, po, 0)
            if b == 0:
                # batch 0 is drain/DMA-paced with little PE work: keep the
                # HAM activity window busy so the clock-gate warms early
                warm_mms(2)
        for pt, et, t in pending_vmuls:
            nc.vector.tensor_mul(pt, et, eb_tiles[t])
        pending_vmuls.clear()
        if prev is not None:
            finish_half(osb, po, 1)
            nc.sync.dma_start(out=out[b - 1], in_=osb)
        prev = (ptiles, vpb)

    # last batch's PV: interleave the two bank chunks per j-tile so each runs
    # as soon as its P tile lands (the all-A-then-all-B order would serialize
    # the whole B pass behind the latest-produced P tile)
    po = ps_o.tile([65, S], f32, tag="po")
    osb = o_pool.tile([65, S], bf16, tag="osb")
    for oi in range(NT):
        mm2(po, prev[0], prev[1], oi, 0)
        mm2(po, prev[0], prev[1], oi, 1)
        if oi == NT - 1:
            finish_half(osb, po, 0)
    finish_half(osb, po, 1)
    nc.sync.dma_start(out=out[B - 1], in_=osb)


def _build_program():
    nc = bacc.Bacc("TRN2", target_bir_lowering=False, debug=False)
    bf16 = mybir.dt.bfloat16
    qt = nc.dram_tensor("qt", [B, 128, S], bf16, kind="ExternalInput").ap()
    kt = nc.dram_tensor("kt", [B, 128, S // 2], bf16, kind="ExternalInput").ap()
    vp = nc.dram_tensor("vp", [B, 128, NT, 66], bf16, kind="ExternalInput").ap()
    bb = nc.dram_tensor("bb", [S, S], bf16, kind="ExternalInput").ap()
    eb = nc.dram_tensor("eb", [S, S], bf16, kind="ExternalInput").ap()
    out = nc.dram_tensor(
        "out", [B, 65, S], bf16, kind="ExternalOutput"
    ).ap()
    with tile.TileContext(nc) as tc, ExitStack() as ctx:
        _emit(ctx, tc, out, qt, kt, vp, bb, eb)
    nc.compile()
    return nc


def get_program():
    global _PROGRAM
    if _PROGRAM is None:
        _PROGRAM = _build_program()
    return _PROGRAM


def make_in_maps(q, k, v, pos_bias):
    import ml_dtypes

    nbf16 = ml_dtypes.bfloat16
    q4 = q.reshape(B, S, H, HD)
    k4 = k.reshape(B, S, H, HD)
    v4 = v.reshape(B, S, H, HD)
    qscale = np.float32(SCALE * A_SCH)
    in_maps = []
    for h in range(H):
        qt = np.empty((B, 128, S), nbf16)
        qt[:, :HD, :] = (q4[:, :, h, :].transpose(0, 2, 1) * qscale).astype(nbf16)
        qt[:, HD:, :] = qt[:, :HD, :]
        # k parity-packed: even tiles on rows 0:64, odd tiles on rows 64:128
        kT = k4[:, :, h, :].transpose(0, 2, 1).astype(nbf16)  # [B, 64, S]
        kt = np.empty((B, 128, S // 2), nbf16)
        for t in range(NT):
            half, idx = t % 2, t // 2
            kt[:, half * 64:(half + 1) * 64, idx * 128:(idx + 1) * 128] = (
                kT[:, :, t * 128:(t + 1) * 128]
            )
        vp = np.concatenate(
            [v4[:, :, h, :], np.ones((B, S, 1), np.float32),
             np.zeros((B, S, 1), np.float32)], axis=2
        )  # [B, S, 66]
        vp = np.ascontiguousarray(
            vp.reshape(B, NT, 128, 66).transpose(0, 2, 1, 3)
        ).astype(nbf16)  # [B, 128, NT, 66]
        btT = np.ascontiguousarray(pos_bias[0, h].T).astype(np.float32)  # [j, i]
        bb = (A_SCH * btT).astype(nbf16)
        eb = np.exp(btT).astype(nbf16)
        in_maps.append({"qt": qt, "kt": kt, "vp": vp, "bb": bb, "eb": eb})
    return in_maps


def assemble_output(results):
    out = np.empty((B, S, D), np.float32)
    for h in range(H):
        o = np.asarray(results[h]["out"], np.float32)  # [B, 65, S]
        normed = o[:, :HD, :] / o[:, HD:HD + 1, :]
        out[:, :, h * HD:(h + 1) * HD] = normed.transpose(0, 2, 1)
    return out


def kernel(q, k, v, pos_bias):
    nc = get_program()
    in_maps = make_in_maps(
        np.asarray(q, np.float32),
        np.asarray(k, np.float32),
        np.asarray(v, np.float32),
        np.asarray(pos_bias, np.float32),
    )
    res = run_bass_kernel_spmd(nc, in_maps, list(range(H))).results
    return assemble_output(res)


# revision 62
# speedup vs baseline: 1.0154x; 1.0083x over previous
"""Multi-head attention with additive positional bias on 8 Trainium2 cores.

Problem: q,k,v [8, 1024, 512] fp32, pos_bias [1, 8, 1024, 1024] fp32,
8 heads x head_dim 64, out = softmax(q@k^T * scale + bias) @ v.

Sharding: one head per NeuronCore (tensor parallel over heads).

Per-core pipeline (scores transposed: S^T[j,i], j on partitions; q is
pre-scaled by A*SCALE on the host so psum holds A*s where A=128/ln2).
The softmax exp drain is a hard constraint (1M psum f32 values per batch
must leave PSUM through ScalarE/VectorE at 1 elem/cycle/lane) and the PE
paces the steady state (~95% busy), so the work is split:
  - PE: QK^T bf16 per pair with column chunks interleaved across the K=64
    row halves [t0c0,t1c0,t0c1,t1c1] so both chunks co-execute; previous
    batch's PV chunks follow each pair's mm1 (mm1 first: ScalarE's exp
    chain needs those psums earliest).
  - VectorE: Schraudolph bit-trick exp for j-tiles 0,2,4,6:
    bitcast_bf16(int16(A*s + 16249 + A*b)) via one scalar_tensor_tensor
    against a resident bf16 A*b table, plus deferred exp(bias) multiplies
    for tiles 5,7 (bf16 tensor_tensor hits the DVE 2x_1P mode, ~683ns;
    emitted after the batch's STTs so the psum drains never queue behind
    ScalarE-dependent work).
  - ScalarE: true exp (free affine scale=1/A) for tiles 1,3,5,7 + the po
    evacuation, split into two 512-col halves (bank A mid-batch, bank B at
    the tail) so only a half-evac sits on the critical loop.
  - GpSimd: exp(bias) multiply for tiles 1,3 (GpSimd TT is ~2.1us/tile;
    these P tiles are consumed late by the next batch's PV).
  - PV: po[dv,i] += [V|ones]^T @ P^T, bank-split: pairs 0-1 accumulate po
    cols 0:512 over all 8 j-tiles (order 0,2,4,6 then 1,3 GpSimd-muled
    then 5,7 Vector-muled, matching P availability), pairs 2-3 cols
    512:1024; 65th row = softmax denominators via the ones column; host
    divides and untransposes.
Even j-tiles on VectorE matter: the 3-deep psum rotation makes mm1(t) wait
on drain(t-3), and even tiles put a Vector STT (not a ScalarE exp that
queues behind the evac) on that edge.
HAM: 8 full-size warmup matmuls during the prologue DMAs + 2 per batch-0
pair keep the PE activity window busy so the clock gate hits 8/8 (~13us)
instead of ~24us; small matmuls do not register as busy.
DMA diet: k is parity-packed [128,512] (even tiles rows 0:64, odd 64:128 -
no duplication), vp carries only 66 columns, output is bf16.
Table DMAs are emitted in first-use order behind batch 0's q/k so the
prologue overlaps compute (the single sync DMA ring is serial; the ACT
ring regressed when tried).
"""

import numpy as np
from contextlib import ExitStack

import concourse.bacc as bacc
import concourse.bass as bass
import concourse.mybir as mybir
import concourse.tile as tile
from concourse.bass_utils import run_bass_kernel_spmd

B = 8          # batch
S = 1024       # sequence length
D = 512        # model dim
H = 8          # heads
HD = 64        # head dim
NT = S // 128  # 128-row j-tiles per sequence
SCALE = HD ** -0.5

A_SCH = 128.0 / np.log(2.0)          # Schraudolph scale (bf16: 2^7 mantissa)
DELTA = -7.0                         # minimax-ish centering of the bit trick
SCH_TILES = (0, 2, 4, 6)             # j-tiles drained by the DVE bit-trick exp
GPS_MUL_TILES = (1, 3)               # exp-tiles whose bias-mul runs on GpSimd
MM2_ORDER = (0, 2, 4, 6, 1, 3, 5, 7)  # PV order: STT tiles first, then the
                                      # GpSimd-muled, then the Vector-muled
                                      # (V muls run after its STTs, latest)
WARMUP_MM = 8                        # full-size PE warmup matmuls during the
                                     # prologue DMA (~3.4us of PE busy: trips
                                     # the HAM activity window so the clock
                                     # gate is 8/8 when real work starts)

_PROGRAM = None


def _emit(ctx, tc, out, qt, kt, vp, bb, eb):
    nc = tc.nc
    f32 = mybir.dt.float32
    bf16 = mybir.dt.bfloat16
    i16 = mybir.dt.int16

    singles = ctx.enter_context(tc.tile_pool(name="singles", bufs=1))
    qk_pool = ctx.enter_context(tc.tile_pool(name="qk_pool", bufs=3))
    v_pool = ctx.enter_context(tc.tile_pool(name="v_pool", bufs=3))
    eg_pool = ctx.enter_context(tc.tile_pool(name="eg_pool", bufs=6))
    ev_pool = ctx.enter_context(tc.tile_pool(name="ev_pool", bufs=6))
    p_pool = ctx.enter_context(tc.tile_pool(name="p_pool", bufs=20))
    o_pool = ctx.enter_context(tc.tile_pool(name="o_pool", bufs=4))
    ps_s = ctx.enter_context(tc.tile_pool(name="ps_s", bufs=3, space="PSUM"))
    ps_o = ctx.enter_context(tc.tile_pool(name="ps_o", bufs=1, space="PSUM"))

    # batch-0 inputs first so compute starts immediately; then tables in
    # first-use order so batch 0's drains are fed as they arrive
    qtb0 = qk_pool.tile([128, S], bf16, name="qtb0", tag="qtb")
    nc.sync.dma_start(out=qtb0, in_=qt[0])
    ktb0 = qk_pool.tile([128, S // 2], bf16, name="ktb0", tag="ktb")
    nc.sync.dma_start(out=ktb0, in_=kt[0])
    bb_tiles = [None] * NT
    eb_tiles = {}
    for t in range(NT):
        if t in SCH_TILES:
            bbt = singles.tile([128, S], bf16, name=f"bbt{t}")
            nc.sync.dma_start(out=bbt, in_=bb[t * 128:(t + 1) * 128, :])
            bb_tiles[t] = bbt
        else:
            ebt = singles.tile([128, S], bf16, name=f"ebt{t}")
            nc.sync.dma_start(out=ebt, in_=eb[t * 128:(t + 1) * 128, :])
            eb_tiles[t] = ebt
    vpb0 = v_pool.tile([128, NT, 66], bf16, name="vpb0", tag="vpb")
    nc.sync.dma_start(out=vpb0, in_=vp[0])

    # PE warmup: FULL-SIZE matmuls (K=128, N=512 — small ones don't register
    # as "busy" to the HAM activity monitor) keep the PE busy during the
    # prologue DMAs and through batch 0 (which has no PV to interleave), so
    # the HAM clock-gate reaches 8/8 before the steady state. The warm psum
    # lives in the ps_o pool: it is unused until batch 1 allocates po, and
    # batch 1's first PV clears the bank (start=True) anyway.
    warm = singles.tile([128, 512 + 128], bf16, name="warm")
    nc.vector.memset(warm, 0.0)
    wps = ps_o.tile([128, S], f32, tag="po")

    def warm_mms(n):
        for _ in range(n):
            nc.tensor.matmul(
                wps[:, 0:512], warm[:, 0:128], warm[:, 128:640],
                start=True, stop=True,
            )

    warm_mms(WARMUP_MM)

    prev = None  # (ptiles, vpb) of previous batch, for interleaved mm2

    def mm1_pair(qtb, ktb, ps0, ps1, t0, t1, serial=False):
        """psum = A*s for j-tiles t0 (rows 0:64) and t1 (rows 64:128).

        Column chunks are interleaved [t0c0, t1c0, t0c1, t1c1]: matmul
        starts are pc-monotone, so same-row-group chunks back-to-back
        would serialize; alternating row groups lets each chunk pair
        co-execute (4 matmuls in ~2 matmul times instead of 3).
        serial=True keeps all of t0 before t1 (batch 0 pair 0: t1's hi-row
        inputs arrive later, and co-execution would head-of-line block t0).
        """
        if serial:
            order = ((t0, ps0, 0), (t0, ps0, 1), (t1, ps1, 0), (t1, ps1, 1))
        else:
            order = ((t0, ps0, 0), (t1, ps1, 0), (t0, ps0, 1), (t1, ps1, 1))
        for t, ps, c in order:
            cs = slice(c * 512, (c + 1) * 512)
            half, idx = t % 2, t // 2
            r = slice(half * 64, (half + 1) * 64)
            nc.tensor.matmul(
                ps[:, cs], ktb[r, idx * 128:(idx + 1) * 128], qtb[r, cs],
                start=True, stop=True,
            )

    pending_vmuls = []  # deferred (pt, et, t): emitted after the batch's STTs

    def drain(ps, t, vmul_all=False):
        """psum -> P tile (bf16 ~ exp(s+b))."""
        pt = p_pool.tile([128, S], bf16, tag="pt")
        if t in SCH_TILES:
            # int16(A*s + (16256-7) + A*b) bitcast to bf16 = ~exp(s+b)
            nc.vector.scalar_tensor_tensor(
                pt.bitcast(i16),
                ps,
                float(127.0 * 128.0 + DELTA),
                bb_tiles[t],
                mybir.AluOpType.add,
                mybir.AluOpType.add,
            )
        else:
            # separate et pools per mul engine: a shared pool would make
            # ScalarE's exp wait on GpSimd's slow muls for tile recycling
            if t in GPS_MUL_TILES and not vmul_all:
                et = eg_pool.tile([128, S], bf16, tag="etg")
                nc.scalar.activation(
                    et, ps, mybir.ActivationFunctionType.Exp,
                    scale=float(1.0 / A_SCH),
                )
                nc.gpsimd.tensor_mul(pt, et, eb_tiles[t])
            else:
                et = ev_pool.tile([128, S], bf16, tag="etv")
                nc.scalar.activation(
                    et, ps, mybir.ActivationFunctionType.Exp,
                    scale=float(1.0 / A_SCH),
                )
                # defer: keeps VectorE's queue free for the psum-draining
                # STTs (a mul here would make them wait on ScalarE's exp)
                pending_vmuls.append((pt, et, t))
        return pt

    def mm2(po, ptiles, vpb, oi, c):
        """One 512-col PV chunk: bank c of po accumulates j-tile MM2_ORDER[oi]."""
        t = MM2_ORDER[oi]
        cs = slice(c * 512, (c + 1) * 512)
        nc.tensor.matmul(
            po[:, cs],
            vpb[:, t, 0:65],
            ptiles[t][:, cs],
            start=(oi == 0),
            stop=(oi == NT - 1),
        )

    def finish_half(osb, po, c):
        cs = slice(c * 512, (c + 1) * 512)
        nc.scalar.activation(
            osb[:, cs], po[:, cs], mybir.ActivationFunctionType.Copy
        )

    for b in range(B):
        if b == 0:
            qtb, ktb, vpb = qtb0, ktb0, vpb0
        else:
            qtb = qk_pool.tile([128, S], bf16, tag="qtb")
            nc.sync.dma_start(out=qtb, in_=qt[b])
            ktb = qk_pool.tile([128, S // 2], bf16, tag="ktb")
            nc.sync.dma_start(out=ktb, in_=kt[b])
            vpb = v_pool.tile([128, NT, 66], bf16, tag="vpb")
            nc.sync.dma_start(out=vpb, in_=vp[b])

        po = None
        osb = None
        if prev is not None:
            po = ps_o.tile([65, S], f32, tag="po")
            osb = o_pool.tile([65, S], bf16, tag="osb")
        ptiles = [None] * NT
        for p in range(NT // 2):
            t0, t1 = 2 * p, 2 * p + 1
            # mm1 first: ScalarE's exp chain is the critical path, and its
            # inputs are these psums — the previous batch's PV fills in after
            ps0 = ps_s.tile([128, S], f32, tag="ps")
            ps1 = ps_s.tile([128, S], f32, tag="ps")
            mm1_pair(qtb, ktb, ps0, ps1, t0, t1)
            if prev is not None:
                # bank-split PV: pairs 0-1 fill po bank A (cols 0:512) for all
                # 8 j-tiles, pairs 2-3 fill bank B — bank A's evacuation runs
                # mid-batch, leaving only a half-evac on the loop tail
                c, base = (0, 4 * p) if p < 2 else (1, 4 * (p - 2))
                for oi in range(base, base + 4):
                    mm2(po, prev[0], prev[1], oi, c)
            ptiles[t0] = drain(ps0, t0)
            ptiles[t1] = drain(ps1, t1)
            if prev is not None and p == 1:
                finish_half(osb, po, 0)
            if b == 0:
                # batch 0 is drain/DMA-paced with little PE work: keep the
                # HAM activity window busy so the clock-gate warms early
                warm_mms(2)
        for pt, et, t in pending_vmuls:
            nc.vector.tensor_mul(pt, et, eb_tiles[t])
        pending_vmuls.clear()
        if prev is not None:
            finish_half(osb, po, 1)
            nc.sync.dma_start(out=out[b - 1], in_=osb)
        prev = (ptiles, vpb)

    # last batch's PV: interleave the two bank chunks per j-tile so each runs
    # as soon as its P tile lands (the all-A-then-all-B order would serialize
    # the whole B pass behind the latest-produced P tile)
    po = ps_o.tile([65, S], f32, tag="po")
    osb = o_pool.tile([65, S], bf16, tag="osb")
    for oi in range(NT):
        mm2(po, prev[0], prev[1], oi, 0)
        mm2(po, prev[0], prev[1], oi, 1)
        if oi == NT - 1:
            finish_half(osb, po, 0)
    # final half-evac split across both psum-capable engines (both idle here)
    nc.scalar.activation(
        osb[:, 512:768], po[:, 512:768], mybir.ActivationFunctionType.Copy
    )
    nc.vector.tensor_copy(osb[:, 768:1024], po[:, 768:1024])
    nc.sync.dma_start(out=out[B - 1], in_=osb)


def _build_program():
    nc = bacc.Bacc("TRN2", target_bir_lowering=False, debug=False)
    bf16 = mybir.dt.bfloat16
    qt = nc.dram_tensor("qt", [B, 128, S], bf16, kind="ExternalInput").ap()
    kt = nc.dram_tensor("kt", [B, 128, S // 2], bf16, kind="ExternalInput").ap()
    vp = nc.dram_tensor("vp", [B, 128, NT, 66], bf16, kind="ExternalInput").ap()
    bb = nc.dram_tensor("bb", [S, S], bf16, kind="ExternalInput").ap()
    eb = nc.dram_tensor("eb", [S, S], bf16, kind="ExternalInput").ap()
    out = nc.dram_tensor(
        "out", [B, 65, S], bf16, kind="ExternalOutput"
    ).ap()
    with tile.TileContext(nc) as tc, ExitStack() as ctx:
        _emit(ctx, tc, out, qt, kt, vp, bb, eb)
    nc.compile()
    return nc


def get_program():
    global _PROGRAM
    if _PROGRAM is None:
        _PROGRAM = _build_program()
    return _PROGRAM


def make_in_maps(q, k, v, pos_bias):
    import ml_dtypes

    nbf16 = ml_dtypes.bfloat16
    q4 = q.reshape(B, S, H, HD)
    k4 = k.reshape(B, S, H, HD)
    v4 = v.reshape(B, S, H, HD)
    qscale = np.float32(SCALE * A_SCH)
    in_maps = []
    for h in range(H):
        qt = np.empty((B, 128, S), nbf16)
        qt[:, :HD, :] = (q4[:, :, h, :].transpose(0, 2, 1) * qscale).astype(nbf16)
        qt[:, HD:, :] = qt[:, :HD, :]
        # k parity-packed: even tiles on rows 0:64, odd tiles on rows 64:128
        kT = k4[:, :, h, :].transpose(0, 2, 1).astype(nbf16)  # [B, 64, S]
        kt = np.empty((B, 128, S // 2), nbf16)
        for t in range(NT):
            half, idx = t % 2, t // 2
            kt[:, half * 64:(half + 1) * 64, idx * 128:(idx + 1) * 128] = (
                kT[:, :, t * 128:(t + 1) * 128]
            )
        vp = np.concatenate(
            [v4[:, :, h, :], np.ones((B, S, 1), np.float32),
             np.zeros((B, S, 1), np.float32)], axis=2
        )  # [B, S, 66]
        vp = np.ascontiguousarray(
            vp.reshape(B, NT, 128, 66).transpose(0, 2, 1, 3)
        ).astype(nbf16)  # [B, 128, NT, 66]
        btT = np.ascontiguousarray(pos_bias[0, h].T).astype(np.float32)  # [j, i]
        bb = (A_SCH * btT).astype(nbf16)
        eb = np.exp(btT).astype(nbf16)
        in_maps.append({"qt": qt, "kt": kt, "vp": vp, "bb": bb, "eb": eb})
    return in_maps


def assemble_output(results):
    out = np.empty((B, S, D), np.float32)
    for h in range(H):
        o = np.asarray(results[h]["out"], np.float32)  # [B, 65, S]
        normed = o[:, :HD, :] / o[:, HD:HD + 1, :]
        out[:, :, h * HD:(h + 1) * HD] = normed.transpose(0, 2, 1)
    return out


def kernel(q, k, v, pos_bias):
    nc = get_program()
    in_maps = make_in_maps(
        np.asarray(q, np.float32),
        np.asarray(k, np.float32),
        np.asarray(v, np.float32),
        np.asarray(pos_bias, np.float32),
    )
    res = run_bass_kernel_spmd(nc, in_maps, list(range(H))).results
    return assemble_output(res)
